# revision 1
# baseline (speedup 1.0000x reference)
"""Trainium2 Bass kernel for CustomEmbeddings (embedding lookup + masked MLP).

Computation (reference):
    emb = emb_table[input_ids]                    # [B, S, D]
    mask = input_ids >= 32000
    h = relu(emb @ w1 + b1); mlp = h @ w2 + b2
    out = where(mask, mlp, emb)

Strategy (8 NeuronCores, SPMD — same program, per-core data):
  - Vocab-parallel table sharding with load-balanced boundaries: the host
    dedups ids (np.unique - each distinct row is gathered exactly once
    device-side; the host unshard scatter replicates rows to duplicate
    tokens at zero extra cost), deals ~U/8 unique ids to each core, and
    ships each core the contiguous vocab range its ids span.  Core c
    gathers its rows (padded to a common static T_cap); the host scatters
    rows back to token positions while unsharding ("shuffle" layout).
    This is the vocab-parallel hint, but the all-reduce is replaced by
    host-side routing, so the device moves each distinct 12.8KB row
    exactly once - ~20% fewer HBM bytes than gathering per-token.
  - The masked-token MLP is tiny (~51 tokens expected, all ids >= 32000 live
    in one 100-row slice of the table which is replicated to every core as a
    small side input).  It is weight-sharded 8-way: core c computes
    h[:, c*800:(c+1)*800] = relu(emb@w1_c + b1_c) and the partial
    mlp_out = h_c @ w2_c.  The 8 partials ([K,3200], ~650KB each) are summed
    on the host during unsharding, + b2, and scattered into masked rows.
"""

import sys

if "/opt/trn_rl_repo" not in sys.path:
    sys.path.insert(0, "/opt/trn_rl_repo")

import numpy as np

from concourse import bacc, bass, mybir
import concourse.tile as tile
from concourse.bass_utils import run_bass_kernel_spmd
from concourse.masks import make_identity

P = 128
VOCAB = 32100
DIM = 3200
HID = 6400
NEW_START = 32000
N_CORES = 8
SHARD_HID = HID // N_CORES          # 800
MLP_TAB_ROWS = P                    # replicated new-token slice, ids-NEW_START < 128
N_K_TILES = DIM // P                # 25


def cdiv(a, b):
    return (a + b - 1) // b


# Testing hook: repeat the main gather loop this many times (same data, same
# outputs) so HW wall-clock scaling can separate device time from dispatch
# overhead.  Always 1 in normal use.
GATHER_REPS = 1


def build_program(n_mlp_chunks: int, n_t_chunks: int, s_rows: int) -> bass.Bass:
    f32 = mybir.dt.float32
    i32 = mybir.dt.int32

    # Bacc (not plain Bass): its finalize() runs the wait-legalization passes
    # (move_matmul_waits_to_ldweights / generate_event_semaphores) that split
    # multi-wait instructions the TRN2 ISA encodings cannot carry.
    nc = bacc.Bacc("TRN2")
    ids_t = nc.declare_dram_parameter("ids_t", [P, n_t_chunks], i32, isOutput=False)
    mlp_ids = nc.declare_dram_parameter(
        "mlp_ids", [P, n_mlp_chunks], i32, isOutput=False
    )
    tshard = nc.declare_dram_parameter("tshard", [s_rows, DIM], f32, isOutput=False)
    mlp_tab = nc.declare_dram_parameter(
        "mlp_tab", [MLP_TAB_ROWS, DIM], f32, isOutput=False
    )
    w1s = nc.declare_dram_parameter("w1s", [DIM, SHARD_HID], f32, isOutput=False)
    b1s = nc.declare_dram_parameter("b1s", [1, SHARD_HID], f32, isOutput=False)
    w2s = nc.declare_dram_parameter("w2s", [SHARD_HID, DIM], f32, isOutput=False)
    out_main = nc.declare_dram_parameter(
        "out_main", [n_t_chunks * P, DIM], f32, isOutput=True
    )
    mlp_part = nc.declare_dram_parameter(
        "mlp_part", [n_mlp_chunks * P, DIM], f32, isOutput=True
    )

    n_hb = cdiv(SHARD_HID, P)  # 7 blocks of h columns (6 full + 32)

    with tile.TileContext(nc) as tc:
        with (
            tc.tile_pool(name="const", bufs=1) as consts,
            tc.tile_pool(name="gpool", bufs=3) as gpool,
            tc.tile_pool(name="mpool", bufs=1) as mpool,
            tc.tile_pool(name="wpool", bufs=2) as wpool,
            tc.tile_pool(name="psA", bufs=2, space="PSUM") as psA,
            tc.tile_pool(name="psH", bufs=1, space="PSUM") as psH,
            tc.tile_pool(name="psO", bufs=1, space="PSUM") as psO,
        ):
            ones_row = consts.tile([1, P], f32)
            nc.gpsimd.memset(ones_row[:], 1.0)
            identity = consts.tile([P, P], f32)
            make_identity(nc, identity[:])
            # Priming transpose: the PE transpose lowers to a pure LW
            # instruction that supports only ONE sync wait.  This op makes PE
            # observe the Pool semaphore (identity/ones memsets), so later
            # transposes only wait on their data input.
            prime = psA.tile([P, P], f32, space="PSUM", tag="tp")
            nc.tensor.transpose(out=prime[:], in_=identity[:], identity=identity[:])

            idx_sb = consts.tile([P, n_t_chunks], i32)
            nc.sync.dma_start(out=idx_sb[:], in_=ids_t[:])
            midx_sb = consts.tile([P, n_mlp_chunks], i32)
            nc.sync.dma_start(out=midx_sb[:], in_=mlp_ids[:])
            b1_sb = consts.tile([1, SHARD_HID], f32)
            nc.sync.dma_start(out=b1_sb[:], in_=b1s[:])

            # ---------------- masked-token MLP (small; overlaps with gather) ----
            for j in range(n_mlp_chunks):
                memb = mpool.tile([P, DIM], f32, tag="memb")
                nc.gpsimd.indirect_dma_start(
                    out=memb[:],
                    out_offset=None,
                    in_=mlp_tab[:],
                    in_offset=bass.IndirectOffsetOnAxis(
                        ap=midx_sb[:, j : j + 1], axis=0
                    ),
                )
                # embT[p, k*P + t] = memb[t, k*P + p]
                embT = mpool.tile([P, DIM], f32, tag="embT")
                for k in range(N_K_TILES):
                    tp = psA.tile([P, P], f32, space="PSUM", tag="tp")
                    nc.tensor.transpose(
                        out=tp[:], in_=memb[:, k * P : (k + 1) * P], identity=identity[:]
                    )
                    nc.vector.tensor_copy(out=embT[:, k * P : (k + 1) * P], in_=tp[:])

                # L1: h = relu(emb @ w1s + b1s), h in [tokens, SHARD_HID]
                hps = psH.tile([P, SHARD_HID], f32, space="PSUM", tag="hps")
                for k in range(N_K_TILES):
                    w1k = wpool.tile([P, SHARD_HID], f32, tag="w1k", bufs=7)
                    nc.sync.dma_start(out=w1k[:], in_=w1s[k * P : (k + 1) * P, :])
                    for n0 in range(0, SHARD_HID, 512):
                        n1 = min(n0 + 512, SHARD_HID)
                        nc.tensor.matmul(
                            hps[:, n0:n1],
                            lhsT=embT[:, k * P : (k + 1) * P],
                            rhs=w1k[:, n0:n1],
                            start=(k == 0),
                            stop=False,
                        )
                # bias add as rank-1 update: ones[tokens] x b1[cols]
                for n0 in range(0, SHARD_HID, 512):
                    n1 = min(n0 + 512, SHARD_HID)
                    nc.tensor.matmul(
                        hps[:, n0:n1],
                        lhsT=ones_row[:1, :],
                        rhs=b1_sb[:1, n0:n1],
                        start=False,
                        stop=True,
                    )
                h_sb = mpool.tile([P, SHARD_HID], f32, tag="h_sb")
                nc.scalar.activation(
                    out=h_sb[:], in_=hps[:], func=mybir.ActivationFunctionType.Relu
                )

                # hT[p, k2*P + t] = h[t, k2*P + p]
                hT = mpool.tile([P, n_hb * P], f32, tag="hT")
                for k2 in range(n_hb):
                    bs = min(P, SHARD_HID - k2 * P)
                    tp2 = psA.tile([P, P], f32, space="PSUM", tag="tp")
                    nc.tensor.transpose(
                        out=tp2[:bs, :],
                        in_=h_sb[:, k2 * P : k2 * P + bs],
                        identity=identity[:],
                    )
                    nc.vector.tensor_copy(
                        out=hT[:bs, k2 * P : (k2 + 1) * P], in_=tp2[:bs, :]
                    )

                # L2 partial: mlp_part = h_c @ w2_c, computed in two column halves
                HALF = DIM // 2  # 1600 -> 4 PSUM banks
                for hh in range(2):
                    c0 = hh * HALF
                    ops = psO.tile([P, HALF], f32, space="PSUM", tag="ops")
                    for k2 in range(n_hb):
                        bs = min(P, SHARD_HID - k2 * P)
                        w2k = wpool.tile([P, HALF], f32, tag="w2k", bufs=8)
                        nc.sync.dma_start(
                            out=w2k[:bs, :],
                            in_=w2s[k2 * P : k2 * P + bs, c0 : c0 + HALF],
                        )
                        for n0 in range(0, HALF, 512):
                            n1 = min(n0 + 512, HALF)
                            nc.tensor.matmul(
                                ops[:, n0:n1],
                                lhsT=hT[:bs, k2 * P : (k2 + 1) * P],
                                rhs=w2k[:bs, n0:n1],
                                start=(k2 == 0),
                                stop=(k2 == n_hb - 1),
                            )
                    ocp = mpool.tile([P, HALF], f32, tag="ocp")
                    nc.vector.tensor_copy(out=ocp[:], in_=ops[:])
                    nc.sync.dma_start(
                        out=mlp_part[j * P : (j + 1) * P, c0 : c0 + HALF], in_=ocp[:]
                    )

            # ---------------- main gather: n_t_chunks*128 rows/core -------------
            for t in [t for _ in range(GATHER_REPS) for t in range(n_t_chunks)]:
                g = gpool.tile([P, DIM], f32, tag="g")
                nc.gpsimd.indirect_dma_start(
                    out=g[:],
                    out_offset=None,
                    in_=tshard[:],
                    in_offset=bass.IndirectOffsetOnAxis(
                        ap=idx_sb[:, t : t + 1], axis=0
                    ),
                )
                nc.sync.dma_start(out=out_main[t * P : (t + 1) * P, :], in_=g[:])

    if not nc.is_finalized():
        nc.finalize()
    return nc


def _wrap(ids, n_chunks):
    """[n_chunks*P] -> [P, n_chunks] with element [p, c] = ids[c*P + p]."""
    return np.ascontiguousarray(ids.reshape(n_chunks, P).T.astype(np.int32))


def _prepare(inputs):
    """Host-side sharding. Returns (n_mlp_chunks, n_t_chunks, in_maps, ctx)."""
    ids = np.asarray(inputs["input_ids"])
    table = np.asarray(inputs["emb_table"], dtype=np.float32)
    w1 = np.asarray(inputs["w1"], dtype=np.float32)
    b1 = np.asarray(inputs["b1"], dtype=np.float32)
    w2 = np.asarray(inputs["w2"], dtype=np.float32)
    b2 = np.asarray(inputs["b2"], dtype=np.float32)

    B, S = ids.shape
    ids_flat = ids.reshape(-1).astype(np.int64)
    N = ids_flat.size

    # --- masked tokens (global; same for every core) ---
    mask = ids_flat >= NEW_START
    masked_pos = np.nonzero(mask)[0]
    K = int(masked_pos.size)
    n_mlp_chunks = max(1, cdiv(K, P))
    mids = np.zeros(n_mlp_chunks * P, dtype=np.int64)
    mids[:K] = ids_flat[masked_pos] - NEW_START
    mlp_ids_t = _wrap(mids, n_mlp_chunks)
    mlp_tab = np.zeros((MLP_TAB_ROWS, DIM), dtype=np.float32)
    mlp_tab[: VOCAB - NEW_START] = table[NEW_START:]

    # --- dedup ids and deal unique rows to cores, exactly balanced ---
    # uniq is sorted; core c gathers uniq[c*per_u : (c+1)*per_u].  Its table
    # shard is the contiguous row range those ids span (ranges may touch at
    # boundaries; overlap in shipped rows is free).
    uniq, inverse = np.unique(ids_flat, return_inverse=True)
    U = int(uniq.size)
    per_u = cdiv(U, N_CORES)
    uniq_per_core = [uniq[c * per_u : (c + 1) * per_u] for c in range(N_CORES)]
    t_counts = [int(u.size) for u in uniq_per_core]
    T_cap = max(P, cdiv(max(t_counts), P) * P)
    n_t_chunks = T_cap // P
    lo_per_core = [int(u[0]) if u.size else 0 for u in uniq_per_core]
    hi_per_core = [int(u[-1]) + 1 if u.size else 1 for u in uniq_per_core]
    s_rows = cdiv(max(h - l for l, h in zip(lo_per_core, hi_per_core)), 16) * 16

    in_maps = []
    for c in range(N_CORES):
        uniq_c = uniq_per_core[c]
        lo = lo_per_core[c]
        hi = min(lo + s_rows, VOCAB)
        loc = np.zeros(T_cap, dtype=np.int64)
        loc[: uniq_c.size] = uniq_c - lo
        tshard = np.zeros((s_rows, DIM), dtype=np.float32)
        tshard[: hi - lo] = table[lo:hi]
        in_maps.append(
            {
                "ids_t": _wrap(loc, n_t_chunks),
                "mlp_ids": mlp_ids_t,
                "tshard": tshard,
                "mlp_tab": mlp_tab,
                "w1s": np.ascontiguousarray(
                    w1[:, c * SHARD_HID : (c + 1) * SHARD_HID]
                ),
                "b1s": np.ascontiguousarray(
                    b1[c * SHARD_HID : (c + 1) * SHARD_HID]
                ).reshape(1, SHARD_HID),
                "w2s": np.ascontiguousarray(
                    w2[c * SHARD_HID : (c + 1) * SHARD_HID, :]
                ),
            }
        )
    ctx = dict(
        B=B, S=S, N=N, masked_pos=masked_pos, K=K, b2=b2,
        inverse=inverse, t_counts=t_counts,
    )
    return n_mlp_chunks, n_t_chunks, s_rows, in_maps, ctx


def _finish(results, ctx):
    allrows = np.concatenate(
        [results[c]["out_main"][: ctx["t_counts"][c]] for c in range(N_CORES)]
    )
    out = allrows[ctx["inverse"]]
    K = ctx["K"]
    if K > 0:
        mlp = results[0]["mlp_part"].astype(np.float32).copy()
        for c in range(1, N_CORES):
            mlp += results[c]["mlp_part"]
        mlp += ctx["b2"][None, :]
        out[ctx["masked_pos"]] = mlp[:K]
    return out.reshape(ctx["B"], ctx["S"], DIM)


def kernel(**inputs) -> np.ndarray:
    n_mlp_chunks, n_t_chunks, s_rows, in_maps, ctx = _prepare(inputs)
    nc = build_program(n_mlp_chunks, n_t_chunks, s_rows)
    res = run_bass_kernel_spmd(nc, in_maps, list(range(N_CORES))).results
    return _finish(res, ctx)



# revision 3
# speedup vs baseline: 1.6368x; 1.6368x over previous
"""Trainium2 Bass kernel for CustomEmbeddings (embedding lookup + masked MLP).

Computation (reference):
    emb = emb_table[input_ids]                    # [B, S, D]
    mask = input_ids >= 32000
    h = relu(emb @ w1 + b1); mlp = h @ w2 + b2
    out = where(mask, mlp, emb)

Strategy (8 NeuronCores, SPMD — same program, per-core data):
  - Token-parallel: core c owns batch row c (2048 tokens).  The host dedups
    each core's ids (np.unique) and ships ONLY the distinct rows its tokens
    touch, packed dense and cast to bf16 ([2048, 3200] bf16 — a 4x byte
    reduction vs shipping a raw f32 vocab slice).  The device performs the
    full embedding lookup: an indirect gather replicates packed rows out to
    all 2048 token positions in token order, so the host unshard is a plain
    concat + cast.  2048 tokens/core bounds the distinct-row count, so the
    static shape is always safe.
  - Masked-token MLP: ids >= 32000 span only 100 possible table rows, so the
    MLP is computed once per TABLE ROW (128-row padded slice, shipped
    pre-transposed), not per token; the host scatters MLP rows to masked
    positions.  It is hidden-sharded 8-way: core c computes
    h_c = relu(emb @ w1[:, c*800:(c+1)*800] + b1_c) directly in transposed
    layout (h_c^T via lhsT=w1 — no on-chip transposes anywhere), then
    partial = h_c @ w2[c*800:(c+1)*800, :].  Host sums the 8 partials, adds
    b2, scatters.  All weights ship in bf16, pre-arranged so every SBUF tile
    is a single contiguous DMA.
"""

import sys

if "/opt/trn_rl_repo" not in sys.path:
    sys.path.insert(0, "/opt/trn_rl_repo")

import ml_dtypes
import numpy as np

from concourse import bacc, bass, mybir
import concourse.tile as tile
from concourse.bass_utils import run_bass_kernel_spmd

P = 128
VOCAB = 32100
DIM = 3200
HID = 6400
NEW_START = 32000
N_CORES = 8
S = 2048                             # tokens per core (= seq len; batch == n_cores)
N_T_CHUNKS = S // P                  # 16 gather chunks
T_CAP = S                            # distinct rows per core is bounded by S
SHARD_HID = HID // N_CORES           # 800
N_K_TILES = DIM // P                 # 25 k-tiles of the MLP input dim
N_H_TILES = (SHARD_HID + P - 1) // P  # 7 hidden tiles (6 full + 32)
MLP_ROWS = P                         # padded new-token table slice (100 real rows)
HALF = DIM // 2                      # L2 output computed in two 1600-col halves

BF16 = ml_dtypes.bfloat16

# Testing hook: repeat the main gather loop this many times (same data, same
# outputs) so HW wall-clock scaling can separate device time from dispatch
# overhead.  Always 1 in normal use.
GATHER_REPS = 1


def build_program() -> bass.Bass:
    f32 = mybir.dt.float32
    bf16 = mybir.dt.bfloat16
    i32 = mybir.dt.int32

    # Bacc (not plain Bass): its finalize() runs the wait-legalization passes
    # that split multi-wait instructions the TRN2 ISA encodings cannot carry.
    nc = bacc.Bacc("TRN2")
    ids_t = nc.declare_dram_parameter("ids_t", [P, N_T_CHUNKS], i32, isOutput=False)
    rows = nc.declare_dram_parameter("rows", [T_CAP, DIM], bf16, isOutput=False)
    # mlp_rowsT[p, k*P + t] = emb_table[NEW_START + t, k*P + p]  (t < 100)
    mlp_rowsT = nc.declare_dram_parameter(
        "mlp_rowsT", [P, DIM], bf16, isOutput=False
    )
    # w1sp[p, k*SHARD_HID + n] = w1[k*P + p, c*SHARD_HID + n]
    w1sp = nc.declare_dram_parameter(
        "w1sp", [P, N_K_TILES * SHARD_HID], bf16, isOutput=False
    )
    b1s = nc.declare_dram_parameter("b1s", [1, SHARD_HID], bf16, isOutput=False)
    # w2sp[p, k2*DIM + n] = w2[c*SHARD_HID + k2*P + p, n]  (zero-padded rows)
    w2sp = nc.declare_dram_parameter(
        "w2sp", [P, N_H_TILES * DIM], bf16, isOutput=False
    )
    out_main = nc.declare_dram_parameter("out_main", [S, DIM], bf16, isOutput=True)
    mlp_part = nc.declare_dram_parameter("mlp_part", [MLP_ROWS, DIM], f32, isOutput=True)

    with tile.TileContext(nc) as tc:
        with (
            tc.tile_pool(name="const", bufs=1) as consts,
            tc.tile_pool(name="gpool", bufs=3) as gpool,
            tc.tile_pool(name="mpool", bufs=1) as mpool,
            tc.tile_pool(name="opool", bufs=2) as opool,
            tc.tile_pool(name="psL1", bufs=2, space="PSUM") as psL1,
            tc.tile_pool(name="psO", bufs=1, space="PSUM") as psO,
        ):
            idx_sb = consts.tile([P, N_T_CHUNKS], i32)
            nc.sync.dma_start(out=idx_sb[:], in_=ids_t[:])

            # ---------------- masked-row MLP (small; overlaps with gather) -----
            embT = mpool.tile([P, DIM], bf16, tag="embT")
            nc.sync.dma_start(out=embT[:], in_=mlp_rowsT[:])
            w1_sb = mpool.tile([P, N_K_TILES * SHARD_HID], bf16, tag="w1_sb")
            nc.sync.dma_start(out=w1_sb[:], in_=w1sp[:])
            b1_sb = consts.tile([1, SHARD_HID], bf16)
            nc.sync.dma_start(out=b1_sb[:], in_=b1s[:])
            ones_row = consts.tile([1, P], bf16)
            nc.gpsimd.memset(ones_row[:], 1.0)
            # hT block c7 holds [hcol partition, token]; partitions >= 32 of the
            # last (32-col) block must be zero, not garbage, because L2 multiplies
            # them by (zero-padded) w2 rows and NaN*0 = NaN.
            hT_sb = mpool.tile([P, N_H_TILES * P], bf16, tag="hT_sb")
            nc.gpsimd.memset(hT_sb[:], 0.0)

            # L1 in transposed layout: hT[c7-block][m=hcol, n=token]
            #   = sum_k w1sp[k-part, m] * mlp_rowsT[k-part, n]  (+ b1 rank-1)
            for c7 in range(N_H_TILES):
                bs = min(P, SHARD_HID - c7 * P)
                hps = psL1.tile([P, P], f32, space="PSUM", tag="hps")
                for k in range(N_K_TILES):
                    nc.tensor.matmul(
                        hps[:bs, :],
                        lhsT=w1_sb[:, k * SHARD_HID + c7 * P : k * SHARD_HID + c7 * P + bs],
                        rhs=embT[:, k * P : (k + 1) * P],
                        start=(k == 0),
                        stop=False,
                    )
                nc.tensor.matmul(
                    hps[:bs, :],
                    lhsT=b1_sb[:1, c7 * P : c7 * P + bs],
                    rhs=ones_row[:1, :],
                    start=False,
                    stop=True,
                )
                nc.scalar.activation(
                    out=hT_sb[:bs, c7 * P : (c7 + 1) * P],
                    in_=hps[:bs, :],
                    func=mybir.ActivationFunctionType.Relu,
                )

            w2_sb = mpool.tile([P, N_H_TILES * DIM], bf16, tag="w2_sb")
            nc.sync.dma_start(out=w2_sb[:], in_=w2sp[:])

            # L2 partial: mlp_part[tok, :] = sum_k2 hT[k2][:, tok]^T @ w2[k2]
            for hh in range(2):
                c0 = hh * HALF
                ops = psO.tile([P, HALF], f32, space="PSUM", tag="ops")
                for k2 in range(N_H_TILES):
                    # 512-wide stripes: matmul outputs may not cross PSUM banks
                    for n0 in range(0, HALF, 512):
                        n1 = min(n0 + 512, HALF)
                        nc.tensor.matmul(
                            ops[:, n0:n1],
                            lhsT=hT_sb[:, k2 * P : (k2 + 1) * P],
                            rhs=w2_sb[:, k2 * DIM + c0 + n0 : k2 * DIM + c0 + n1],
                            start=(k2 == 0),
                            stop=(k2 == N_H_TILES - 1),
                        )
                ocp = opool.tile([P, HALF], f32, tag="ocp")
                nc.vector.tensor_copy(out=ocp[:], in_=ops[:])
                nc.sync.dma_start(
                    out=mlp_part[:, c0 : c0 + HALF], in_=ocp[:]
                )

            # ---------------- main lookup: replicate rows to token order -------
            for t in [t for _ in range(GATHER_REPS) for t in range(N_T_CHUNKS)]:
                g = gpool.tile([P, DIM], bf16, tag="g")
                nc.gpsimd.indirect_dma_start(
                    out=g[:],
                    out_offset=None,
                    in_=rows[:],
                    in_offset=bass.IndirectOffsetOnAxis(
                        ap=idx_sb[:, t : t + 1], axis=0
                    ),
                )
                nc.sync.dma_start(out=out_main[t * P : (t + 1) * P, :], in_=g[:])

    if not nc.is_finalized():
        nc.finalize()
    return nc


def _wrap(ids, n_chunks):
    """[n_chunks*P] -> [P, n_chunks] with element [p, c] = ids[c*P + p]."""
    return np.ascontiguousarray(ids.reshape(n_chunks, P).T.astype(np.int32))


def _prepare(inputs):
    """Host-side sharding. Returns (in_maps, ctx)."""
    ids = np.asarray(inputs["input_ids"])
    table = np.asarray(inputs["emb_table"], dtype=np.float32)
    w1 = np.asarray(inputs["w1"], dtype=np.float32)
    b1 = np.asarray(inputs["b1"], dtype=np.float32)
    w2 = np.asarray(inputs["w2"], dtype=np.float32)
    b2 = np.asarray(inputs["b2"], dtype=np.float32)

    B, S_in = ids.shape
    assert B == N_CORES and S_in == S, (ids.shape,)
    assert table.shape == (VOCAB, DIM)

    # new-token slice for the MLP, pre-transposed: [p, k*P + t]
    n_new = VOCAB - NEW_START
    mlp_rows = np.zeros((MLP_ROWS, DIM), dtype=np.float32)
    mlp_rows[:n_new] = table[NEW_START:]
    mlp_rowsT = (
        mlp_rows.reshape(MLP_ROWS, N_K_TILES, P)
        .transpose(2, 1, 0)
        .reshape(P, N_K_TILES * MLP_ROWS)
        .astype(BF16)
    )

    w1b = w1.astype(BF16)
    w2b = w2.astype(BF16)
    b1b = b1.astype(BF16)

    in_maps = []
    for c in range(N_CORES):
        uniq, inv = np.unique(ids[c].astype(np.int64), return_inverse=True)
        rows = np.zeros((T_CAP, DIM), dtype=BF16)
        rows[: uniq.size] = table[uniq].astype(BF16)
        # w1sp[p, k*SHARD_HID + n] = w1[k*P + p, c*SHARD_HID + n]
        w1sp = np.ascontiguousarray(
            w1b[:, c * SHARD_HID : (c + 1) * SHARD_HID]
            .reshape(N_K_TILES, P, SHARD_HID)
            .transpose(1, 0, 2)
            .reshape(P, N_K_TILES * SHARD_HID)
        )
        # w2sp[p, k2*DIM + n] = w2[c*SHARD_HID + k2*P + p, n], zero-padded
        w2pad = np.zeros((N_H_TILES * P, DIM), dtype=BF16)
        w2pad[:SHARD_HID] = w2b[c * SHARD_HID : (c + 1) * SHARD_HID]
        w2sp = np.ascontiguousarray(
            w2pad.reshape(N_H_TILES, P, DIM).transpose(1, 0, 2).reshape(P, N_H_TILES * DIM)
        )
        in_maps.append(
            {
                "ids_t": _wrap(inv.astype(np.int64), N_T_CHUNKS),
                "rows": rows,
                "mlp_rowsT": mlp_rowsT,
                "w1sp": w1sp,
                "b1s": np.ascontiguousarray(
                    b1b[c * SHARD_HID : (c + 1) * SHARD_HID]
                ).reshape(1, SHARD_HID),
                "w2sp": w2sp,
            }
        )
    ctx = dict(ids=ids, b2=b2)
    return in_maps, ctx


def _finish(results, ctx):
    ids = ctx["ids"]
    out = np.empty((N_CORES * S, DIM), dtype=np.float32)
    for c in range(N_CORES):
        out[c * S : (c + 1) * S] = results[c]["out_main"].astype(np.float32)
    ids_flat = ids.reshape(-1).astype(np.int64)
    masked_pos = np.nonzero(ids_flat >= NEW_START)[0]
    if masked_pos.size:
        mlp = results[0]["mlp_part"].astype(np.float32).copy()
        for c in range(1, N_CORES):
            mlp += results[c]["mlp_part"]
        mlp += ctx["b2"][None, :]
        out[masked_pos] = mlp[ids_flat[masked_pos] - NEW_START]
    return out.reshape(N_CORES, S, DIM)


def kernel(**inputs) -> np.ndarray:
    in_maps, ctx = _prepare(inputs)
    nc = build_program()
    res = run_bass_kernel_spmd(nc, in_maps, list(range(N_CORES))).results
    return _finish(res, ctx)


# revision 4
# speedup vs baseline: 2.3489x; 1.4350x over previous
"""Trainium2 Bass kernel for CustomEmbeddings (embedding lookup + masked MLP).

Computation (reference):
    emb = emb_table[input_ids]                    # [B, S, D]
    mask = input_ids >= 32000
    h = relu(emb @ w1 + b1); mlp = h @ w2 + b2
    out = where(mask, mlp, emb)

Strategy (8 NeuronCores, SPMD — same program, per-core data):
  - Token-parallel: core c owns batch row c (2048 tokens).  The host dedups
    each core's ids (np.unique) and ships ONLY the distinct rows its tokens
    touch, packed dense and quantized to int8 with one f32 scale per row
    (max|row|/127; scales stay on the host).  The device performs the full
    embedding lookup: an indirect gather replicates packed rows out to all
    2048 token positions in token order; the host unshard is a dequantize
    (q * scale[token]) + concat.  Per-row int8 keeps max quantization error
    at ~4e-4 abs (3.8e-3 of output scale) and cuts the dominant staged /
    gathered / written bytes 4x vs f32.  2048 tokens/core bounds the
    distinct-row count, so the static shape is always safe.
  - Masked-token MLP: ids >= 32000 span only 100 possible table rows, so the
    MLP is computed once per TABLE ROW (128-row padded slice, shipped
    pre-transposed in bf16), not per token; the host scatters MLP rows to
    masked positions.  It is hidden-sharded 8-way: core c computes
    h_c = relu(emb @ w1[:, c*800:(c+1)*800] + b1_c) directly in transposed
    layout (h_c^T via lhsT=w1 — no on-chip transposes anywhere), then
    partial = h_c @ w2[c*800:(c+1)*800, :] in f32 PSUM.  Host sums the 8
    partials, adds b2, scatters.  Weights ship in bf16, pre-arranged so
    every SBUF tile is a single contiguous DMA.
"""

import sys

if "/opt/trn_rl_repo" not in sys.path:
    sys.path.insert(0, "/opt/trn_rl_repo")

import ml_dtypes
import numpy as np

from concourse import bacc, bass, mybir
import concourse.tile as tile
from concourse.bass_utils import run_bass_kernel_spmd

P = 128
VOCAB = 32100
DIM = 3200
HID = 6400
NEW_START = 32000
N_CORES = 8
S = 2048                             # tokens per core (= seq len; batch == n_cores)
N_T_CHUNKS = S // P                  # 16 gather chunks
T_CAP = S                            # distinct rows per core is bounded by S
SHARD_HID = HID // N_CORES           # 800
N_K_TILES = DIM // P                 # 25 k-tiles of the MLP input dim
N_H_TILES = (SHARD_HID + P - 1) // P  # 7 hidden tiles (6 full + 32)
MLP_ROWS = P                         # padded new-token table slice (100 real rows)
HALF = DIM // 2                      # L2 output computed in two 1600-col halves

BF16 = ml_dtypes.bfloat16

# Testing hook: repeat the main gather loop this many times (same data, same
# outputs) so HW wall-clock scaling can separate device time from dispatch
# overhead.  Always 1 in normal use.
GATHER_REPS = 1


def build_program() -> bass.Bass:
    f32 = mybir.dt.float32
    bf16 = mybir.dt.bfloat16
    i8 = mybir.dt.int8
    i32 = mybir.dt.int32

    # Bacc (not plain Bass): its finalize() runs the wait-legalization passes
    # that split multi-wait instructions the TRN2 ISA encodings cannot carry.
    nc = bacc.Bacc("TRN2")
    ids_t = nc.declare_dram_parameter("ids_t", [P, N_T_CHUNKS], i32, isOutput=False)
    rows = nc.declare_dram_parameter("rows", [T_CAP, DIM], i8, isOutput=False)
    # mlp_rowsT[p, k*P + t] = emb_table[NEW_START + t, k*P + p]  (t < 100)
    mlp_rowsT = nc.declare_dram_parameter(
        "mlp_rowsT", [P, DIM], bf16, isOutput=False
    )
    # w1sp[p, k*SHARD_HID + n] = w1[k*P + p, c*SHARD_HID + n]
    w1sp = nc.declare_dram_parameter(
        "w1sp", [P, N_K_TILES * SHARD_HID], bf16, isOutput=False
    )
    b1s = nc.declare_dram_parameter("b1s", [1, SHARD_HID], bf16, isOutput=False)
    # w2sp[p, k2*DIM + n] = w2[c*SHARD_HID + k2*P + p, n]  (zero-padded rows)
    w2sp = nc.declare_dram_parameter(
        "w2sp", [P, N_H_TILES * DIM], bf16, isOutput=False
    )
    out_main = nc.declare_dram_parameter("out_main", [S, DIM], i8, isOutput=True)
    mlp_part = nc.declare_dram_parameter("mlp_part", [MLP_ROWS, DIM], f32, isOutput=True)

    with tile.TileContext(nc) as tc:
        with (
            tc.tile_pool(name="const", bufs=1) as consts,
            tc.tile_pool(name="gpool", bufs=3) as gpool,
            tc.tile_pool(name="mpool", bufs=1) as mpool,
            tc.tile_pool(name="opool", bufs=2) as opool,
            tc.tile_pool(name="psL1", bufs=2, space="PSUM") as psL1,
            tc.tile_pool(name="psO", bufs=1, space="PSUM") as psO,
        ):
            idx_sb = consts.tile([P, N_T_CHUNKS], i32)
            nc.sync.dma_start(out=idx_sb[:], in_=ids_t[:])

            # ---------------- masked-row MLP (small; overlaps with gather) -----
            embT = mpool.tile([P, DIM], bf16, tag="embT")
            nc.sync.dma_start(out=embT[:], in_=mlp_rowsT[:])
            w1_sb = mpool.tile([P, N_K_TILES * SHARD_HID], bf16, tag="w1_sb")
            nc.sync.dma_start(out=w1_sb[:], in_=w1sp[:])
            b1_sb = consts.tile([1, SHARD_HID], bf16)
            nc.sync.dma_start(out=b1_sb[:], in_=b1s[:])
            ones_row = consts.tile([1, P], bf16)
            nc.gpsimd.memset(ones_row[:], 1.0)
            # hT block c7 holds [hcol partition, token]; partitions >= 32 of the
            # last (32-col) block must be zero, not garbage, because L2 multiplies
            # them by (zero-padded) w2 rows and NaN*0 = NaN.
            hT_sb = mpool.tile([P, N_H_TILES * P], bf16, tag="hT_sb")
            nc.gpsimd.memset(hT_sb[:], 0.0)

            # L1 in transposed layout: hT[c7-block][m=hcol, n=token]
            #   = sum_k w1sp[k-part, m] * mlp_rowsT[k-part, n]  (+ b1 rank-1)
            for c7 in range(N_H_TILES):
                bs = min(P, SHARD_HID - c7 * P)
                hps = psL1.tile([P, P], f32, space="PSUM", tag="hps")
                for k in range(N_K_TILES):
                    nc.tensor.matmul(
                        hps[:bs, :],
                        lhsT=w1_sb[:, k * SHARD_HID + c7 * P : k * SHARD_HID + c7 * P + bs],
                        rhs=embT[:, k * P : (k + 1) * P],
                        start=(k == 0),
                        stop=False,
                    )
                nc.tensor.matmul(
                    hps[:bs, :],
                    lhsT=b1_sb[:1, c7 * P : c7 * P + bs],
                    rhs=ones_row[:1, :],
                    start=False,
                    stop=True,
                )
                nc.scalar.activation(
                    out=hT_sb[:bs, c7 * P : (c7 + 1) * P],
                    in_=hps[:bs, :],
                    func=mybir.ActivationFunctionType.Relu,
                )

            w2_sb = mpool.tile([P, N_H_TILES * DIM], bf16, tag="w2_sb")
            nc.sync.dma_start(out=w2_sb[:], in_=w2sp[:])

            # L2 partial: mlp_part[tok, :] = sum_k2 hT[k2][:, tok]^T @ w2[k2]
            for hh in range(2):
                c0 = hh * HALF
                ops = psO.tile([P, HALF], f32, space="PSUM", tag="ops")
                for k2 in range(N_H_TILES):
                    # 512-wide stripes: matmul outputs may not cross PSUM banks
                    for n0 in range(0, HALF, 512):
                        n1 = min(n0 + 512, HALF)
                        nc.tensor.matmul(
                            ops[:, n0:n1],
                            lhsT=hT_sb[:, k2 * P : (k2 + 1) * P],
                            rhs=w2_sb[:, k2 * DIM + c0 + n0 : k2 * DIM + c0 + n1],
                            start=(k2 == 0),
                            stop=(k2 == N_H_TILES - 1),
                        )
                ocp = opool.tile([P, HALF], f32, tag="ocp")
                nc.vector.tensor_copy(out=ocp[:], in_=ops[:])
                nc.sync.dma_start(
                    out=mlp_part[:, c0 : c0 + HALF], in_=ocp[:]
                )

            # ---------------- main lookup: replicate rows to token order -------
            for t in [t for _ in range(GATHER_REPS) for t in range(N_T_CHUNKS)]:
                g = gpool.tile([P, DIM], i8, tag="g")
                nc.gpsimd.indirect_dma_start(
                    out=g[:],
                    out_offset=None,
                    in_=rows[:],
                    in_offset=bass.IndirectOffsetOnAxis(
                        ap=idx_sb[:, t : t + 1], axis=0
                    ),
                )
                nc.sync.dma_start(out=out_main[t * P : (t + 1) * P, :], in_=g[:])

    if not nc.is_finalized():
        nc.finalize()
    return nc


def _wrap(ids, n_chunks):
    """[n_chunks*P] -> [P, n_chunks] with element [p, c] = ids[c*P + p]."""
    return np.ascontiguousarray(ids.reshape(n_chunks, P).T.astype(np.int32))


def _prepare(inputs):
    """Host-side sharding. Returns (in_maps, ctx)."""
    ids = np.asarray(inputs["input_ids"])
    table = np.asarray(inputs["emb_table"], dtype=np.float32)
    w1 = np.asarray(inputs["w1"], dtype=np.float32)
    b1 = np.asarray(inputs["b1"], dtype=np.float32)
    w2 = np.asarray(inputs["w2"], dtype=np.float32)
    b2 = np.asarray(inputs["b2"], dtype=np.float32)

    B, S_in = ids.shape
    assert B == N_CORES and S_in == S, (ids.shape,)
    assert table.shape == (VOCAB, DIM)

    # new-token slice for the MLP, pre-transposed: [p, k*P + t]
    n_new = VOCAB - NEW_START
    mlp_rows = np.zeros((MLP_ROWS, DIM), dtype=np.float32)
    mlp_rows[:n_new] = table[NEW_START:]
    mlp_rowsT = (
        mlp_rows.reshape(MLP_ROWS, N_K_TILES, P)
        .transpose(2, 1, 0)
        .reshape(P, N_K_TILES * MLP_ROWS)
        .astype(BF16)
    )

    w1b = w1.astype(BF16)
    w2b = w2.astype(BF16)
    b1b = b1.astype(BF16)

    in_maps = []
    scales = []
    invs = []
    for c in range(N_CORES):
        uniq, inv = np.unique(ids[c].astype(np.int64), return_inverse=True)
        packed = table[uniq]                              # [U, DIM] f32
        s = np.abs(packed).max(axis=1) / 127.0            # per-row scale
        s = np.maximum(s, 1e-30)
        q = np.clip(np.rint(packed / s[:, None]), -127, 127).astype(np.int8)
        rows = np.zeros((T_CAP, DIM), dtype=np.int8)
        rows[: uniq.size] = q
        sc = np.ones(T_CAP, dtype=np.float32)
        sc[: uniq.size] = s
        scales.append(sc)
        invs.append(inv.astype(np.int64))
        # w1sp[p, k*SHARD_HID + n] = w1[k*P + p, c*SHARD_HID + n]
        w1sp = np.ascontiguousarray(
            w1b[:, c * SHARD_HID : (c + 1) * SHARD_HID]
            .reshape(N_K_TILES, P, SHARD_HID)
            .transpose(1, 0, 2)
            .reshape(P, N_K_TILES * SHARD_HID)
        )
        # w2sp[p, k2*DIM + n] = w2[c*SHARD_HID + k2*P + p, n], zero-padded
        w2pad = np.zeros((N_H_TILES * P, DIM), dtype=BF16)
        w2pad[:SHARD_HID] = w2b[c * SHARD_HID : (c + 1) * SHARD_HID]
        w2sp = np.ascontiguousarray(
            w2pad.reshape(N_H_TILES, P, DIM).transpose(1, 0, 2).reshape(P, N_H_TILES * DIM)
        )
        in_maps.append(
            {
                "ids_t": _wrap(inv.astype(np.int64), N_T_CHUNKS),
                "rows": rows,
                "mlp_rowsT": mlp_rowsT,
                "w1sp": w1sp,
                "b1s": np.ascontiguousarray(
                    b1b[c * SHARD_HID : (c + 1) * SHARD_HID]
                ).reshape(1, SHARD_HID),
                "w2sp": w2sp,
            }
        )
    ctx = dict(ids=ids, b2=b2, scales=scales, invs=invs)
    return in_maps, ctx


def _finish(results, ctx):
    ids = ctx["ids"]
    out = np.empty((N_CORES * S, DIM), dtype=np.float32)
    for c in range(N_CORES):
        # dequantize: token t's row was quantized with scale[inv[t]]
        tok_scale = ctx["scales"][c][ctx["invs"][c]]      # [S]
        out[c * S : (c + 1) * S] = (
            results[c]["out_main"].astype(np.float32) * tok_scale[:, None]
        )
    ids_flat = ids.reshape(-1).astype(np.int64)
    masked_pos = np.nonzero(ids_flat >= NEW_START)[0]
    if masked_pos.size:
        mlp = results[0]["mlp_part"].astype(np.float32).copy()
        for c in range(1, N_CORES):
            mlp += results[c]["mlp_part"]
        mlp += ctx["b2"][None, :]
        out[masked_pos] = mlp[ids_flat[masked_pos] - NEW_START]
    return out.reshape(N_CORES, S, DIM)


def kernel(**inputs) -> np.ndarray:
    in_maps, ctx = _prepare(inputs)
    nc = build_program()
    res = run_bass_kernel_spmd(nc, in_maps, list(range(N_CORES))).results
    return _finish(res, ctx)


# revision 6
# speedup vs baseline: 2.8844x; 1.2280x over previous
"""Trainium2 Bass kernel for CustomEmbeddings (embedding lookup + masked MLP).

Computation (reference):
    emb = emb_table[input_ids]                    # [B, S, D]
    mask = input_ids >= 32000
    h = relu(emb @ w1 + b1); mlp = h @ w2 + b2
    out = where(mask, mlp, emb)

Strategy (8 NeuronCores, SPMD — same program, per-core data):
  - Token-parallel: core c owns batch row c (2048 tokens).  The host dedups
    each core's ids (np.unique) and ships ONLY the distinct rows its tokens
    touch, packed dense and quantized to int8 with one f32 scale per row
    (max|row|/127; scales stay on the host).  The device performs the full
    embedding lookup: an indirect gather replicates packed rows out to all
    2048 token positions in token order; the host unshard is a dequantize
    (q * scale[token]) + concat.  Per-row int8 keeps max quantization error
    at ~4e-4 abs (3.9e-3 of output scale) and cuts the dominant staged /
    gathered / written bytes 4x vs f32.  2048 tokens/core bounds the
    distinct-row count, so the static shape is always safe.
  - Masked-token MLP: ids >= 32000 span only 100 possible table rows, so the
    MLP is computed once per TABLE ROW (128-row padded slice, shipped
    pre-transposed in bf16), not per token; the host scatters MLP rows to
    masked positions.  It is hidden-sharded 8-way: core c computes
    h_c = relu(emb @ w1[:, c*800:(c+1)*800] + b1_c) directly in transposed
    layout (h_c^T via lhsT=w1 — no on-chip transposes anywhere), then
    partial = h_c @ w2[c*800:(c+1)*800, :] in f32 PSUM.  Host sums the 8
    partials, adds b2, scatters.
  - Weights ship as int8 with per-column f32 scales: int8 values cast to
    bf16 on-chip (exact — integers <= 127), the w1 scale & true b1 fold into
    the ReLU activation's per-partition scale/bias (relu(x)*s = relu(x*s)
    for s > 0), and the w2 per-column scale is applied by the host on each
    core's f32 partial before summing.  The only losses are the int8
    quantization itself (~0.9% rms per layer) and bf16 h/emb rounding;
    measured end-to-end rel err is 7.1e-3 vs the 2e-2 gate.
"""

import sys

if "/opt/trn_rl_repo" not in sys.path:
    sys.path.insert(0, "/opt/trn_rl_repo")

import ml_dtypes
import numpy as np

from concourse import bacc, bass, mybir
import concourse.tile as tile
from concourse.bass_utils import run_bass_kernel_spmd

P = 128
VOCAB = 32100
DIM = 3200
HID = 6400
NEW_START = 32000
N_CORES = 8
S = 2048                             # tokens per core (= seq len; batch == n_cores)
N_T_CHUNKS = S // P                  # 16 gather chunks
T_CAP = S                            # distinct rows per core is bounded by S
SHARD_HID = HID // N_CORES           # 800
N_K_TILES = DIM // P                 # 25 k-tiles of the MLP input dim
N_H_TILES = (SHARD_HID + P - 1) // P  # 7 hidden tiles (6 full + 32)
MLP_ROWS = P                         # padded new-token table slice (100 real rows)
HALF = DIM // 2                      # L2 output computed in two 1600-col halves

BF16 = ml_dtypes.bfloat16

# Testing hook: repeat the main gather loop this many times (same data, same
# outputs) so HW wall-clock scaling can separate device time from dispatch
# overhead.  Always 1 in normal use.
GATHER_REPS = 1


def build_program() -> bass.Bass:
    f32 = mybir.dt.float32
    bf16 = mybir.dt.bfloat16
    i8 = mybir.dt.int8
    i32 = mybir.dt.int32

    # Bacc (not plain Bass): its finalize() runs the wait-legalization passes
    # that split multi-wait instructions the TRN2 ISA encodings cannot carry.
    nc = bacc.Bacc("TRN2")
    ids_t = nc.declare_dram_parameter("ids_t", [P, N_T_CHUNKS], i32, isOutput=False)
    rows = nc.declare_dram_parameter("rows", [T_CAP, DIM], i8, isOutput=False)
    # mlp_rowsT[p, k*P + t] = emb_table[NEW_START + t, k*P + p]  (t < 100)
    mlp_rowsT = nc.declare_dram_parameter(
        "mlp_rowsT", [P, DIM], bf16, isOutput=False
    )
    # w1q[p, k*SHARD_HID + n] = int8 of w1[k*P + p, c*SHARD_HID + n] / s1[n]
    w1q = nc.declare_dram_parameter(
        "w1q", [P, N_K_TILES * SHARD_HID], i8, isOutput=False
    )
    # per-hidden-col dequant scale and true bias, laid out per c7 block:
    # s1b[p, c7] = s1[c7*P + p], b1b[p, c7] = b1[c*SHARD_HID + c7*P + p]
    s1b = nc.declare_dram_parameter("s1b", [P, N_H_TILES], f32, isOutput=False)
    b1b = nc.declare_dram_parameter("b1b", [P, N_H_TILES], f32, isOutput=False)
    # w2q[p, k2*DIM + n] = int8 of w2[c*SHARD_HID + k2*P + p, n] / s2[n]
    w2q = nc.declare_dram_parameter(
        "w2q", [P, N_H_TILES * DIM], i8, isOutput=False
    )
    out_main = nc.declare_dram_parameter("out_main", [S, DIM], i8, isOutput=True)
    # raw partial (pre s2-scale); host multiplies by s2 and sums across cores
    mlp_part = nc.declare_dram_parameter("mlp_part", [MLP_ROWS, DIM], f32, isOutput=True)

    with tile.TileContext(nc) as tc:
        with (
            tc.tile_pool(name="const", bufs=1) as consts,
            tc.tile_pool(name="gpool", bufs=3) as gpool,
            tc.tile_pool(name="mpool", bufs=1) as mpool,
            tc.tile_pool(name="opool", bufs=2) as opool,
            tc.tile_pool(name="psL1", bufs=2, space="PSUM") as psL1,
            tc.tile_pool(name="psO", bufs=1, space="PSUM") as psO,
        ):
            idx_sb = consts.tile([P, N_T_CHUNKS], i32)
            nc.sync.dma_start(out=idx_sb[:], in_=ids_t[:])

            # ---------------- masked-row MLP (small; overlaps with gather) -----
            embT = mpool.tile([P, DIM], bf16, tag="embT")
            nc.sync.dma_start(out=embT[:], in_=mlp_rowsT[:])
            w1q_sb = mpool.tile([P, N_K_TILES * SHARD_HID], i8, tag="w1q_sb")
            nc.sync.dma_start(out=w1q_sb[:], in_=w1q[:])
            w1_sb = mpool.tile([P, N_K_TILES * SHARD_HID], bf16, tag="w1_sb")
            # exact cast: |q| <= 127 is representable in bf16
            nc.vector.tensor_copy(out=w1_sb[:], in_=w1q_sb[:])
            s1_sb = consts.tile([P, N_H_TILES], f32)
            nc.sync.dma_start(out=s1_sb[:], in_=s1b[:])
            b1_sb = consts.tile([P, N_H_TILES], f32)
            nc.sync.dma_start(out=b1_sb[:], in_=b1b[:])
            # hT block c7 holds [hcol partition, token]; partitions >= 32 of the
            # last (32-col) block must be zero, not garbage, because L2 multiplies
            # them by (zero-padded) w2 rows and NaN*0 = NaN.
            hT_sb = mpool.tile([P, N_H_TILES * P], bf16, tag="hT_sb")
            nc.gpsimd.memset(hT_sb[:], 0.0)

            # L1 in transposed layout: raw[c7-block][m=hcol, n=token]
            #   = sum_k w1q[k-part, m] * mlp_rowsT[k-part, n]
            # then h = relu(raw * s1 + b1) via the activation's per-partition
            # scale/bias (relu(x*s) = relu(x)*s for s > 0 makes this exact).
            for c7 in range(N_H_TILES):
                bs = min(P, SHARD_HID - c7 * P)
                hps = psL1.tile([P, P], f32, space="PSUM", tag="hps")
                for k in range(N_K_TILES):
                    nc.tensor.matmul(
                        hps[:bs, :],
                        lhsT=w1_sb[:, k * SHARD_HID + c7 * P : k * SHARD_HID + c7 * P + bs],
                        rhs=embT[:, k * P : (k + 1) * P],
                        start=(k == 0),
                        stop=(k == N_K_TILES - 1),
                    )
                nc.scalar.activation(
                    out=hT_sb[:bs, c7 * P : (c7 + 1) * P],
                    in_=hps[:bs, :],
                    func=mybir.ActivationFunctionType.Relu,
                    scale=s1_sb[:bs, c7 : c7 + 1],
                    bias=b1_sb[:bs, c7 : c7 + 1],
                )

            w2q_sb = mpool.tile([P, N_H_TILES * DIM], i8, tag="w2q_sb")
            nc.sync.dma_start(out=w2q_sb[:], in_=w2q[:])
            w2_sb = mpool.tile([P, N_H_TILES * DIM], bf16, tag="w2_sb")
            nc.vector.tensor_copy(out=w2_sb[:], in_=w2q_sb[:])

            # L2 raw partial: mlp_part[tok, :] = sum_k2 hT[k2][:, tok]^T @ w2q[k2]
            for hh in range(2):
                c0 = hh * HALF
                ops = psO.tile([P, HALF], f32, space="PSUM", tag="ops")
                for k2 in range(N_H_TILES):
                    # 512-wide stripes: matmul outputs may not cross PSUM banks
                    for n0 in range(0, HALF, 512):
                        n1 = min(n0 + 512, HALF)
                        nc.tensor.matmul(
                            ops[:, n0:n1],
                            lhsT=hT_sb[:, k2 * P : (k2 + 1) * P],
                            rhs=w2_sb[:, k2 * DIM + c0 + n0 : k2 * DIM + c0 + n1],
                            start=(k2 == 0),
                            stop=(k2 == N_H_TILES - 1),
                        )
                ocp = opool.tile([P, HALF], f32, tag="ocp")
                nc.vector.tensor_copy(out=ocp[:], in_=ops[:])
                nc.sync.dma_start(
                    out=mlp_part[:, c0 : c0 + HALF], in_=ocp[:]
                )

            # ---------------- main lookup: replicate rows to token order -------
            for t in [t for _ in range(GATHER_REPS) for t in range(N_T_CHUNKS)]:
                g = gpool.tile([P, DIM], i8, tag="g")
                nc.gpsimd.indirect_dma_start(
                    out=g[:],
                    out_offset=None,
                    in_=rows[:],
                    in_offset=bass.IndirectOffsetOnAxis(
                        ap=idx_sb[:, t : t + 1], axis=0
                    ),
                )
                nc.sync.dma_start(out=out_main[t * P : (t + 1) * P, :], in_=g[:])

    if not nc.is_finalized():
        nc.finalize()
    return nc


def _wrap(ids, n_chunks):
    """[n_chunks*P] -> [P, n_chunks] with element [p, c] = ids[c*P + p]."""
    return np.ascontiguousarray(ids.reshape(n_chunks, P).T.astype(np.int32))


def _quant_cols(w):
    """Per-column symmetric int8: returns (q [r, c] int8, s [c] f32)."""
    s = np.abs(w).max(axis=0) / 127.0
    s = np.maximum(s, 1e-30).astype(np.float32)
    q = np.clip(np.rint(w / s[None, :]), -127, 127).astype(np.int8)
    return q, s


def _prepare(inputs):
    """Host-side sharding. Returns (in_maps, ctx)."""
    ids = np.asarray(inputs["input_ids"])
    table = np.asarray(inputs["emb_table"], dtype=np.float32)
    w1 = np.asarray(inputs["w1"], dtype=np.float32)
    b1 = np.asarray(inputs["b1"], dtype=np.float32)
    w2 = np.asarray(inputs["w2"], dtype=np.float32)
    b2 = np.asarray(inputs["b2"], dtype=np.float32)

    B, S_in = ids.shape
    assert B == N_CORES and S_in == S, (ids.shape,)
    assert table.shape == (VOCAB, DIM)

    # new-token slice for the MLP, pre-transposed: [p, k*P + t]
    n_new = VOCAB - NEW_START
    mlp_rows = np.zeros((MLP_ROWS, DIM), dtype=np.float32)
    mlp_rows[:n_new] = table[NEW_START:]
    mlp_rowsT = (
        mlp_rows.reshape(MLP_ROWS, N_K_TILES, P)
        .transpose(2, 1, 0)
        .reshape(P, N_K_TILES * MLP_ROWS)
        .astype(BF16)
    )

    in_maps = []
    scales = []
    invs = []
    s2s = []
    for c in range(N_CORES):
        uniq, inv = np.unique(ids[c].astype(np.int64), return_inverse=True)
        packed = table[uniq]                              # [U, DIM] f32
        s = np.abs(packed).max(axis=1) / 127.0            # per-row scale
        s = np.maximum(s, 1e-30)
        q = np.clip(np.rint(packed / s[:, None]), -127, 127).astype(np.int8)
        rows = np.zeros((T_CAP, DIM), dtype=np.int8)
        rows[: uniq.size] = q
        sc = np.ones(T_CAP, dtype=np.float32)
        sc[: uniq.size] = s
        scales.append(sc)
        invs.append(inv.astype(np.int64))

        w1s = w1[:, c * SHARD_HID : (c + 1) * SHARD_HID]  # [DIM, SHARD_HID]
        w1qs, s1 = _quant_cols(w1s)
        # w1q[p, k*SHARD_HID + n] = w1qs[k*P + p, n]
        w1qp = np.ascontiguousarray(
            w1qs.reshape(N_K_TILES, P, SHARD_HID)
            .transpose(1, 0, 2)
            .reshape(P, N_K_TILES * SHARD_HID)
        )
        s1pad = np.ones(N_H_TILES * P, dtype=np.float32)
        s1pad[:SHARD_HID] = s1
        b1pad = np.zeros(N_H_TILES * P, dtype=np.float32)
        b1pad[:SHARD_HID] = b1[c * SHARD_HID : (c + 1) * SHARD_HID]
        s1b = np.ascontiguousarray(s1pad.reshape(N_H_TILES, P).T)
        b1b = np.ascontiguousarray(b1pad.reshape(N_H_TILES, P).T)

        w2s = w2[c * SHARD_HID : (c + 1) * SHARD_HID, :]  # [SHARD_HID, DIM]
        w2qs, s2 = _quant_cols(w2s)
        s2s.append(s2)
        w2pad = np.zeros((N_H_TILES * P, DIM), dtype=np.int8)
        w2pad[:SHARD_HID] = w2qs
        w2qp = np.ascontiguousarray(
            w2pad.reshape(N_H_TILES, P, DIM).transpose(1, 0, 2).reshape(P, N_H_TILES * DIM)
        )
        in_maps.append(
            {
                "ids_t": _wrap(inv.astype(np.int64), N_T_CHUNKS),
                "rows": rows,
                "mlp_rowsT": mlp_rowsT,
                "w1q": w1qp,
                "s1b": s1b,
                "b1b": b1b,
                "w2q": w2qp,
            }
        )
    ctx = dict(ids=ids, b2=b2, scales=scales, invs=invs, s2s=s2s)
    return in_maps, ctx


def _finish(results, ctx):
    ids = ctx["ids"]
    out = np.empty((N_CORES * S, DIM), dtype=np.float32)
    for c in range(N_CORES):
        # dequantize: token t's row was quantized with scale[inv[t]]
        tok_scale = ctx["scales"][c][ctx["invs"][c]]      # [S]
        out[c * S : (c + 1) * S] = (
            results[c]["out_main"].astype(np.float32) * tok_scale[:, None]
        )
    ids_flat = ids.reshape(-1).astype(np.int64)
    masked_pos = np.nonzero(ids_flat >= NEW_START)[0]
    if masked_pos.size:
        mlp = results[0]["mlp_part"].astype(np.float32) * ctx["s2s"][0][None, :]
        for c in range(1, N_CORES):
            mlp += results[c]["mlp_part"] * ctx["s2s"][c][None, :]
        mlp += ctx["b2"][None, :]
        out[masked_pos] = mlp[ids_flat[masked_pos] - NEW_START]
    return out.reshape(N_CORES, S, DIM)


def kernel(**inputs) -> np.ndarray:
    in_maps, ctx = _prepare(inputs)
    nc = build_program()
    res = run_bass_kernel_spmd(nc, in_maps, list(range(N_CORES))).results
    return _finish(res, ctx)


# revision 8
# speedup vs baseline: 2.9287x; 1.0153x over previous
"""Trainium2 Bass kernel for CustomEmbeddings (embedding lookup + masked MLP).

Computation (reference):
    emb = emb_table[input_ids]                    # [B, S, D]
    mask = input_ids >= 32000
    h = relu(emb @ w1 + b1); mlp = h @ w2 + b2
    out = where(mask, mlp, emb)

Strategy (8 NeuronCores, SPMD — same program, per-core data):
  - Token-parallel: core c owns batch row c (2048 tokens).  The host dedups
    each core's ids (np.unique) and ships ONLY the distinct rows its tokens
    touch, packed dense and quantized to int8 with one f32 scale per row
    (max|row|/127; scales stay on the host).  The device performs the full
    embedding lookup: an indirect gather replicates packed rows out to all
    2048 token positions in token order; the host unshard is a dequantize
    (q * scale[token]) + concat.  Per-row int8 keeps max quantization error
    at ~4e-4 abs (3.9e-3 of output scale) and cuts the dominant staged /
    gathered / written bytes 4x vs f32.  2048 tokens/core bounds the
    distinct-row count, so the static shape is always safe.
  - Masked-token MLP: ids >= 32000 span only 100 possible table rows, so the
    MLP is computed once per TABLE ROW (128-row padded slice, shipped
    pre-transposed in bf16), not per token; the host scatters MLP rows to
    masked positions.  It is hidden-sharded 8-way: core c computes
    h_c = relu(emb @ w1[:, c*800:(c+1)*800] + b1_c) directly in transposed
    layout (h_c^T via lhsT=w1 — no on-chip transposes anywhere), then
    partial = h_c @ w2[c*800:(c+1)*800, :] in f32 PSUM.  Host sums the 8
    partials, adds b2, scatters.
  - Weights ship as int8 with per-column f32 scales: int8 values cast to
    bf16 on-chip (exact — integers <= 127), the w1 scale & true b1 fold into
    the ReLU activation's per-partition scale/bias (relu(x)*s = relu(x*s)
    for s > 0), and the w2 per-column scale is applied by the host on each
    core's f32 partial before summing.  The only losses are the int8
    quantization itself (~0.9% rms per layer) and bf16 h/emb rounding;
    measured end-to-end rel err is 7.1e-3 vs the 2e-2 gate.
"""

import sys

if "/opt/trn_rl_repo" not in sys.path:
    sys.path.insert(0, "/opt/trn_rl_repo")

import ml_dtypes
import numpy as np

from concourse import bacc, bass, mybir
import concourse.tile as tile
from concourse.bass_utils import run_bass_kernel_spmd

P = 128
VOCAB = 32100
DIM = 3200
HID = 6400
NEW_START = 32000
N_CORES = 8
S = 2048                             # tokens per core (= seq len; batch == n_cores)
N_T_CHUNKS = S // P                  # 16 gather chunks
T_CAP = S                            # distinct rows per core is bounded by S
SHARD_HID = HID // N_CORES           # 800
N_K_TILES = DIM // P                 # 25 k-tiles of the MLP input dim
N_H_TILES = (SHARD_HID + P - 1) // P  # 7 hidden tiles (6 full + 32)
MLP_ROWS = P                         # padded new-token table slice (100 real rows)
HALF = DIM // 2                      # L2 output computed in two 1600-col halves

BF16 = ml_dtypes.bfloat16

# Testing hook: repeat the main gather loop this many times (same data, same
# outputs) so HW wall-clock scaling can separate device time from dispatch
# overhead.  Always 1 in normal use.
GATHER_REPS = 1


def build_program() -> bass.Bass:
    f32 = mybir.dt.float32
    bf16 = mybir.dt.bfloat16
    i8 = mybir.dt.int8
    i32 = mybir.dt.int32

    # Bacc (not plain Bass): its finalize() runs the wait-legalization passes
    # that split multi-wait instructions the TRN2 ISA encodings cannot carry.
    nc = bacc.Bacc("TRN2")
    ids_t = nc.declare_dram_parameter("ids_t", [P, N_T_CHUNKS], i32, isOutput=False)
    rows = nc.declare_dram_parameter("rows", [T_CAP, DIM], i8, isOutput=False)
    # mlp_rowsT[p, k*P + t] = emb_table[NEW_START + t, k*P + p]  (t < 100)
    mlp_rowsT = nc.declare_dram_parameter(
        "mlp_rowsT", [P, DIM], bf16, isOutput=False
    )
    # w1q[p, k*SHARD_HID + n] = int8 of w1[k*P + p, c*SHARD_HID + n] / s1[n]
    w1q = nc.declare_dram_parameter(
        "w1q", [P, N_K_TILES * SHARD_HID], i8, isOutput=False
    )
    # per-hidden-col dequant scale and true bias, laid out per c7 block:
    # s1b[p, c7] = s1[c7*P + p], b1b[p, c7] = b1[c*SHARD_HID + c7*P + p]
    s1b = nc.declare_dram_parameter("s1b", [P, N_H_TILES], f32, isOutput=False)
    b1b = nc.declare_dram_parameter("b1b", [P, N_H_TILES], f32, isOutput=False)
    # w2q[p, k2*DIM + n] = int8 of w2[c*SHARD_HID + k2*P + p, n] / s2[n]
    w2q = nc.declare_dram_parameter(
        "w2q", [P, N_H_TILES * DIM], i8, isOutput=False
    )
    out_main = nc.declare_dram_parameter("out_main", [S, DIM], i8, isOutput=True)
    # raw partial (pre s2-scale); host multiplies by s2 and sums across cores.
    # Only the 100 real new-token rows are emitted (rows 100-127 are padding).
    n_new = VOCAB - NEW_START
    mlp_part = nc.declare_dram_parameter("mlp_part", [n_new, DIM], f32, isOutput=True)

    with tile.TileContext(nc) as tc:
        with (
            tc.tile_pool(name="const", bufs=1) as consts,
            tc.tile_pool(name="gpool", bufs=3) as gpool,
            tc.tile_pool(name="mpool", bufs=1) as mpool,
            tc.tile_pool(name="opool", bufs=2) as opool,
            tc.tile_pool(name="psL1", bufs=2, space="PSUM") as psL1,
            tc.tile_pool(name="psO", bufs=1, space="PSUM") as psO,
        ):
            idx_sb = consts.tile([P, N_T_CHUNKS], i32)
            nc.sync.dma_start(out=idx_sb[:], in_=ids_t[:])

            # ---------------- masked-row MLP (small; overlaps with gather) -----
            embT = mpool.tile([P, DIM], bf16, tag="embT")
            nc.sync.dma_start(out=embT[:], in_=mlp_rowsT[:])
            w1q_sb = mpool.tile([P, N_K_TILES * SHARD_HID], i8, tag="w1q_sb")
            nc.sync.dma_start(out=w1q_sb[:], in_=w1q[:])
            w1_sb = mpool.tile([P, N_K_TILES * SHARD_HID], bf16, tag="w1_sb")
            # exact cast: |q| <= 127 is representable in bf16
            nc.vector.tensor_copy(out=w1_sb[:], in_=w1q_sb[:])
            s1_sb = consts.tile([P, N_H_TILES], f32)
            nc.sync.dma_start(out=s1_sb[:], in_=s1b[:])
            b1_sb = consts.tile([P, N_H_TILES], f32)
            nc.sync.dma_start(out=b1_sb[:], in_=b1b[:])
            # hT block c7 holds [hcol partition, token]; partitions >= 32 of the
            # last (32-col) block must be zero, not garbage, because L2 multiplies
            # them by (zero-padded) w2 rows and NaN*0 = NaN.
            hT_sb = mpool.tile([P, N_H_TILES * P], bf16, tag="hT_sb")
            nc.gpsimd.memset(hT_sb[:], 0.0)

            # L1 in transposed layout: raw[c7-block][m=hcol, n=token]
            #   = sum_k w1q[k-part, m] * mlp_rowsT[k-part, n]
            # then h = relu(raw * s1 + b1) via the activation's per-partition
            # scale/bias (relu(x*s) = relu(x)*s for s > 0 makes this exact).
            for c7 in range(N_H_TILES):
                bs = min(P, SHARD_HID - c7 * P)
                hps = psL1.tile([P, P], f32, space="PSUM", tag="hps")
                for k in range(N_K_TILES):
                    nc.tensor.matmul(
                        hps[:bs, :],
                        lhsT=w1_sb[:, k * SHARD_HID + c7 * P : k * SHARD_HID + c7 * P + bs],
                        rhs=embT[:, k * P : (k + 1) * P],
                        start=(k == 0),
                        stop=(k == N_K_TILES - 1),
                    )
                nc.scalar.activation(
                    out=hT_sb[:bs, c7 * P : (c7 + 1) * P],
                    in_=hps[:bs, :],
                    func=mybir.ActivationFunctionType.Relu,
                    scale=s1_sb[:bs, c7 : c7 + 1],
                    bias=b1_sb[:bs, c7 : c7 + 1],
                )

            w2q_sb = mpool.tile([P, N_H_TILES * DIM], i8, tag="w2q_sb")
            nc.sync.dma_start(out=w2q_sb[:], in_=w2q[:])
            w2_sb = mpool.tile([P, N_H_TILES * DIM], bf16, tag="w2_sb")
            nc.vector.tensor_copy(out=w2_sb[:], in_=w2q_sb[:])

            # L2 raw partial: mlp_part[tok, :] = sum_k2 hT[k2][:, tok]^T @ w2q[k2]
            for hh in range(2):
                c0 = hh * HALF
                ops = psO.tile([P, HALF], f32, space="PSUM", tag="ops")
                for k2 in range(N_H_TILES):
                    # 512-wide stripes: matmul outputs may not cross PSUM banks
                    for n0 in range(0, HALF, 512):
                        n1 = min(n0 + 512, HALF)
                        nc.tensor.matmul(
                            ops[:, n0:n1],
                            lhsT=hT_sb[:, k2 * P : (k2 + 1) * P],
                            rhs=w2_sb[:, k2 * DIM + c0 + n0 : k2 * DIM + c0 + n1],
                            start=(k2 == 0),
                            stop=(k2 == N_H_TILES - 1),
                        )
                ocp = opool.tile([P, HALF], f32, tag="ocp")
                nc.vector.tensor_copy(out=ocp[:], in_=ops[:])
                nc.sync.dma_start(
                    out=mlp_part[:, c0 : c0 + HALF], in_=ocp[:n_new, :]
                )

            # ---------------- main lookup: replicate rows to token order -------
            for t in [t for _ in range(GATHER_REPS) for t in range(N_T_CHUNKS)]:
                g = gpool.tile([P, DIM], i8, tag="g")
                nc.gpsimd.indirect_dma_start(
                    out=g[:],
                    out_offset=None,
                    in_=rows[:],
                    in_offset=bass.IndirectOffsetOnAxis(
                        ap=idx_sb[:, t : t + 1], axis=0
                    ),
                )
                nc.sync.dma_start(out=out_main[t * P : (t + 1) * P, :], in_=g[:])

    if not nc.is_finalized():
        nc.finalize()
    return nc


def _wrap(ids, n_chunks):
    """[n_chunks*P] -> [P, n_chunks] with element [p, c] = ids[c*P + p]."""
    return np.ascontiguousarray(ids.reshape(n_chunks, P).T.astype(np.int32))


def _quant_cols(w):
    """Per-column symmetric int8: returns (q [r, c] int8, s [c] f32)."""
    s = np.abs(w).max(axis=0) / 127.0
    s = np.maximum(s, 1e-30).astype(np.float32)
    q = np.clip(np.rint(w / s[None, :]), -127, 127).astype(np.int8)
    return q, s


def _prepare(inputs):
    """Host-side sharding. Returns (in_maps, ctx)."""
    ids = np.asarray(inputs["input_ids"])
    table = np.asarray(inputs["emb_table"], dtype=np.float32)
    w1 = np.asarray(inputs["w1"], dtype=np.float32)
    b1 = np.asarray(inputs["b1"], dtype=np.float32)
    w2 = np.asarray(inputs["w2"], dtype=np.float32)
    b2 = np.asarray(inputs["b2"], dtype=np.float32)

    B, S_in = ids.shape
    assert B == N_CORES and S_in == S, (ids.shape,)
    assert table.shape == (VOCAB, DIM)

    # new-token slice for the MLP, pre-transposed: [p, k*P + t]
    n_new = VOCAB - NEW_START
    mlp_rows = np.zeros((MLP_ROWS, DIM), dtype=np.float32)
    mlp_rows[:n_new] = table[NEW_START:]
    mlp_rowsT = (
        mlp_rows.reshape(MLP_ROWS, N_K_TILES, P)
        .transpose(2, 1, 0)
        .reshape(P, N_K_TILES * MLP_ROWS)
        .astype(BF16)
    )

    in_maps = []
    scales = []
    invs = []
    s2s = []
    for c in range(N_CORES):
        uniq, inv = np.unique(ids[c].astype(np.int64), return_inverse=True)
        packed = table[uniq]                              # [U, DIM] f32
        s = np.abs(packed).max(axis=1) / 127.0            # per-row scale
        s = np.maximum(s, 1e-30)
        q = np.clip(np.rint(packed / s[:, None]), -127, 127).astype(np.int8)
        rows = np.zeros((T_CAP, DIM), dtype=np.int8)
        rows[: uniq.size] = q
        sc = np.ones(T_CAP, dtype=np.float32)
        sc[: uniq.size] = s
        scales.append(sc)
        invs.append(inv.astype(np.int64))

        w1s = w1[:, c * SHARD_HID : (c + 1) * SHARD_HID]  # [DIM, SHARD_HID]
        w1qs, s1 = _quant_cols(w1s)
        # w1q[p, k*SHARD_HID + n] = w1qs[k*P + p, n]
        w1qp = np.ascontiguousarray(
            w1qs.reshape(N_K_TILES, P, SHARD_HID)
            .transpose(1, 0, 2)
            .reshape(P, N_K_TILES * SHARD_HID)
        )
        s1pad = np.ones(N_H_TILES * P, dtype=np.float32)
        s1pad[:SHARD_HID] = s1
        b1pad = np.zeros(N_H_TILES * P, dtype=np.float32)
        b1pad[:SHARD_HID] = b1[c * SHARD_HID : (c + 1) * SHARD_HID]
        s1b = np.ascontiguousarray(s1pad.reshape(N_H_TILES, P).T)
        b1b = np.ascontiguousarray(b1pad.reshape(N_H_TILES, P).T)

        w2s = w2[c * SHARD_HID : (c + 1) * SHARD_HID, :]  # [SHARD_HID, DIM]
        w2qs, s2 = _quant_cols(w2s)
        s2s.append(s2)
        w2pad = np.zeros((N_H_TILES * P, DIM), dtype=np.int8)
        w2pad[:SHARD_HID] = w2qs
        w2qp = np.ascontiguousarray(
            w2pad.reshape(N_H_TILES, P, DIM).transpose(1, 0, 2).reshape(P, N_H_TILES * DIM)
        )
        in_maps.append(
            {
                "ids_t": _wrap(inv.astype(np.int64), N_T_CHUNKS),
                "rows": rows,
                "mlp_rowsT": mlp_rowsT,
                "w1q": w1qp,
                "s1b": s1b,
                "b1b": b1b,
                "w2q": w2qp,
            }
        )
    ctx = dict(ids=ids, b2=b2, scales=scales, invs=invs, s2s=s2s)
    return in_maps, ctx


def _finish(results, ctx):
    ids = ctx["ids"]
    out = np.empty((N_CORES * S, DIM), dtype=np.float32)
    for c in range(N_CORES):
        # dequantize: token t's row was quantized with scale[inv[t]]
        tok_scale = ctx["scales"][c][ctx["invs"][c]]      # [S]
        out[c * S : (c + 1) * S] = (
            results[c]["out_main"].astype(np.float32) * tok_scale[:, None]
        )
    ids_flat = ids.reshape(-1).astype(np.int64)
    masked_pos = np.nonzero(ids_flat >= NEW_START)[0]
    if masked_pos.size:
        mlp = results[0]["mlp_part"].astype(np.float32) * ctx["s2s"][0][None, :]
        for c in range(1, N_CORES):
            mlp += results[c]["mlp_part"] * ctx["s2s"][c][None, :]
        mlp += ctx["b2"][None, :]
        out[masked_pos] = mlp[ids_flat[masked_pos] - NEW_START]
    return out.reshape(N_CORES, S, DIM)


def kernel(**inputs) -> np.ndarray:
    in_maps, ctx = _prepare(inputs)
    nc = build_program()
    res = run_bass_kernel_spmd(nc, in_maps, list(range(N_CORES))).results
    return _finish(res, ctx)


# revision 15
# speedup vs baseline: 2.9810x; 1.0179x over previous
"""Trainium2 Bass kernel for CustomEmbeddings (embedding lookup + masked MLP).

Computation (reference):
    emb = emb_table[input_ids]                    # [B, S, D]
    mask = input_ids >= 32000
    h = relu(emb @ w1 + b1); mlp = h @ w2 + b2
    out = where(mask, mlp, emb)

Strategy (8 NeuronCores, SPMD — same program, per-core data):
  - Token-parallel: core c owns batch row c (2048 tokens).  The host dedups
    each core's ids (np.unique) and ships ONLY the distinct rows its tokens
    touch, packed dense and quantized to int8 with one f32 scale per row
    (max|row|/127; scales stay on the host).  The device performs the full
    embedding lookup: an indirect gather replicates packed rows out to all
    2048 token positions in token order; the host unshard is a dequantize
    (q * scale[token]) + concat.  Per-row int8 keeps max quantization error
    at ~4e-4 abs (3.9e-3 of output scale) and cuts the dominant staged /
    gathered / written bytes 4x vs f32.  2048 tokens/core bounds the
    distinct-row count, so the static shape is always safe.
  - Masked-token MLP: ids >= 32000 span only 100 possible table rows, so the
    MLP is computed once per TABLE ROW (128-row padded slice, shipped
    pre-transposed in bf16), not per token; the host scatters MLP rows to
    masked positions.  It is hidden-sharded 8-way: core c computes
    h_c = relu(emb @ w1[:, c*800:(c+1)*800] + b1_c) directly in transposed
    layout (h_c^T via lhsT=w1 — no on-chip transposes anywhere), then
    partial = h_c @ w2[c*800:(c+1)*800, :] in f32 PSUM.  Host sums the 8
    partials, adds b2, scatters.
  - Weights ship as int8 with per-column f32 scales: int8 values cast to
    bf16 on-chip (exact — integers <= 127), the w1 scale & true b1 fold into
    the ReLU activation's per-partition scale/bias (relu(x)*s = relu(x*s)
    for s > 0), and the w2 per-column scale is applied by the host on each
    core's f32 partial before summing.  The only losses are the int8
    quantization itself (~0.9% rms per layer) and bf16 h/emb rounding;
    measured end-to-end rel err is 7.1e-3 vs the 2e-2 gate.
"""

import sys

if "/opt/trn_rl_repo" not in sys.path:
    sys.path.insert(0, "/opt/trn_rl_repo")

import ml_dtypes
import numpy as np

from concourse import bacc, bass, mybir
import concourse.tile as tile
from concourse.bass_utils import run_bass_kernel_spmd

P = 128
VOCAB = 32100
DIM = 3200
HID = 6400
NEW_START = 32000
N_CORES = 8
S = 2048                             # tokens per core (= seq len; batch == n_cores)
N_T_CHUNKS = S // P                  # 16 gather chunks
T_CAP = S                            # distinct rows per core is bounded by S
SHARD_HID = HID // N_CORES           # 800
N_K_TILES = DIM // P                 # 25 k-tiles of the MLP input dim
N_H_TILES = (SHARD_HID + P - 1) // P  # 7 hidden tiles (6 full + 32)
MLP_ROWS = P                         # padded new-token table slice (100 real rows)
HALF = DIM // 2                      # L2 output computed in two 1600-col halves

BF16 = ml_dtypes.bfloat16

# Testing hook: repeat the main gather loop this many times (same data, same
# outputs) so HW wall-clock scaling can separate device time from dispatch
# overhead.  Always 1 in normal use.
GATHER_REPS = 1


def build_program(emb_i8: bool = False) -> bass.Bass:
    """emb_i8: ship the MLP input slice int8 (per-row scales folded out on the
    host).  Only valid when b1 == 0 (relu(x/s) = relu(x)/s needs no bias in
    the scaled domain); the caller picks the variant from the data."""
    f32 = mybir.dt.float32
    bf16 = mybir.dt.bfloat16
    i8 = mybir.dt.int8
    i32 = mybir.dt.int32

    # Bacc (not plain Bass): its finalize() runs the wait-legalization passes
    # that split multi-wait instructions the TRN2 ISA encodings cannot carry.
    nc = bacc.Bacc("TRN2")
    ids_t = nc.declare_dram_parameter("ids_t", [P, N_T_CHUNKS], i32, isOutput=False)
    rows = nc.declare_dram_parameter("rows", [T_CAP, DIM], i8, isOutput=False)
    # mlp_rowsT[p, k*P + t] = emb_table[NEW_START + t, k*P + p]  (t < 100)
    mlp_rowsT = nc.declare_dram_parameter(
        "mlp_rowsT", [P, DIM], i8 if emb_i8 else bf16, isOutput=False
    )
    # w1q[p, k*SHARD_HID + n] = int8 of w1[k*P + p, c*SHARD_HID + n] / s1[n]
    w1q = nc.declare_dram_parameter(
        "w1q", [P, N_K_TILES * SHARD_HID], i8, isOutput=False
    )
    # per-hidden-col dequant scale and true bias, laid out per c7 block:
    # s1b[p, c7] = s1[c7*P + p], b1b[p, c7] = b1[c*SHARD_HID + c7*P + p]
    s1b = nc.declare_dram_parameter("s1b", [P, N_H_TILES], f32, isOutput=False)
    b1b = nc.declare_dram_parameter("b1b", [P, N_H_TILES], f32, isOutput=False)
    # w2q[p, k2*DIM + n] = int8 of w2[c*SHARD_HID + k2*P + p, n] / s2[n]
    w2q = nc.declare_dram_parameter(
        "w2q", [P, N_H_TILES * DIM], i8, isOutput=False
    )
    out_main = nc.declare_dram_parameter("out_main", [S, DIM], i8, isOutput=True)
    # raw partial (pre s2-scale); host multiplies by s2 and sums across cores.
    # Only the 100 real new-token rows are emitted (rows 100-127 are padding).
    n_new = VOCAB - NEW_START
    mlp_part = nc.declare_dram_parameter("mlp_part", [n_new, DIM], f32, isOutput=True)

    with tile.TileContext(nc) as tc:
        with (
            tc.tile_pool(name="const", bufs=1) as consts,
            tc.tile_pool(name="gpool", bufs=3) as gpool,
            tc.tile_pool(name="mpool", bufs=1) as mpool,
            tc.tile_pool(name="opool", bufs=2) as opool,
            tc.tile_pool(name="psL1", bufs=2, space="PSUM") as psL1,
            tc.tile_pool(name="psO", bufs=1, space="PSUM") as psO,
        ):
            idx_sb = consts.tile([P, N_T_CHUNKS], i32)
            nc.sync.dma_start(out=idx_sb[:], in_=ids_t[:])

            # ---------------- masked-row MLP (small; overlaps with gather) -----
            if emb_i8:
                embq_sb = mpool.tile([P, DIM], i8, tag="embq_sb")
                nc.sync.dma_start(out=embq_sb[:], in_=mlp_rowsT[:])
                embT = mpool.tile([P, DIM], bf16, tag="embT")
                nc.vector.tensor_copy(out=embT[:], in_=embq_sb[:])
            else:
                embT = mpool.tile([P, DIM], bf16, tag="embT")
                nc.sync.dma_start(out=embT[:], in_=mlp_rowsT[:])
            w1q_sb = mpool.tile([P, N_K_TILES * SHARD_HID], i8, tag="w1q_sb")
            nc.sync.dma_start(out=w1q_sb[:], in_=w1q[:])
            w1_sb = mpool.tile([P, N_K_TILES * SHARD_HID], bf16, tag="w1_sb")
            # exact cast: |q| <= 127 is representable in bf16
            nc.vector.tensor_copy(out=w1_sb[:], in_=w1q_sb[:])
            s1_sb = consts.tile([P, N_H_TILES], f32)
            nc.sync.dma_start(out=s1_sb[:], in_=s1b[:])
            b1_sb = consts.tile([P, N_H_TILES], f32)
            nc.sync.dma_start(out=b1_sb[:], in_=b1b[:])
            # hT block c7 holds [hcol partition, token]; partitions >= 32 of the
            # last (32-col) block must be zero, not garbage, because L2 multiplies
            # them by (zero-padded) w2 rows and NaN*0 = NaN.
            hT_sb = mpool.tile([P, N_H_TILES * P], bf16, tag="hT_sb")
            nc.gpsimd.memset(hT_sb[:], 0.0)

            # L1 in transposed layout: raw[c7-block][m=hcol, n=token]
            #   = sum_k w1q[k-part, m] * mlp_rowsT[k-part, n]
            # then h = relu(raw * s1 + b1) via the activation's per-partition
            # scale/bias (relu(x*s) = relu(x)*s for s > 0 makes this exact).
            for c7 in range(N_H_TILES):
                bs = min(P, SHARD_HID - c7 * P)
                hps = psL1.tile([P, P], f32, space="PSUM", tag="hps")
                for k in range(N_K_TILES):
                    nc.tensor.matmul(
                        hps[:bs, :],
                        lhsT=w1_sb[:, k * SHARD_HID + c7 * P : k * SHARD_HID + c7 * P + bs],
                        rhs=embT[:, k * P : (k + 1) * P],
                        start=(k == 0),
                        stop=(k == N_K_TILES - 1),
                    )
                nc.scalar.activation(
                    out=hT_sb[:bs, c7 * P : (c7 + 1) * P],
                    in_=hps[:bs, :],
                    func=mybir.ActivationFunctionType.Relu,
                    scale=s1_sb[:bs, c7 : c7 + 1],
                    bias=b1_sb[:bs, c7 : c7 + 1],
                )

            w2q_sb = mpool.tile([P, N_H_TILES * DIM], i8, tag="w2q_sb")
            nc.sync.dma_start(out=w2q_sb[:], in_=w2q[:])
            w2_sb = mpool.tile([P, N_H_TILES * DIM], bf16, tag="w2_sb")
            nc.vector.tensor_copy(out=w2_sb[:], in_=w2q_sb[:])

            # L2 raw partial: mlp_part[tok, :] = sum_k2 hT[k2][:, tok]^T @ w2q[k2]
            for hh in range(2):
                c0 = hh * HALF
                ops = psO.tile([P, HALF], f32, space="PSUM", tag="ops")
                for k2 in range(N_H_TILES):
                    # 512-wide stripes: matmul outputs may not cross PSUM banks
                    for n0 in range(0, HALF, 512):
                        n1 = min(n0 + 512, HALF)
                        nc.tensor.matmul(
                            ops[:, n0:n1],
                            lhsT=hT_sb[:, k2 * P : (k2 + 1) * P],
                            rhs=w2_sb[:, k2 * DIM + c0 + n0 : k2 * DIM + c0 + n1],
                            start=(k2 == 0),
                            stop=(k2 == N_H_TILES - 1),
                        )
                ocp = opool.tile([P, HALF], f32, tag="ocp")
                nc.vector.tensor_copy(out=ocp[:], in_=ops[:])
                nc.sync.dma_start(
                    out=mlp_part[:, c0 : c0 + HALF], in_=ocp[:n_new, :]
                )

            # ---------------- main lookup: replicate rows to token order -------
            for t in [t for _ in range(GATHER_REPS) for t in range(N_T_CHUNKS)]:
                g = gpool.tile([P, DIM], i8, tag="g")
                nc.gpsimd.indirect_dma_start(
                    out=g[:],
                    out_offset=None,
                    in_=rows[:],
                    in_offset=bass.IndirectOffsetOnAxis(
                        ap=idx_sb[:, t : t + 1], axis=0
                    ),
                )
                nc.sync.dma_start(out=out_main[t * P : (t + 1) * P, :], in_=g[:])

    if not nc.is_finalized():
        nc.finalize()
    return nc


def _wrap(ids, n_chunks):
    """[n_chunks*P] -> [P, n_chunks] with element [p, c] = ids[c*P + p]."""
    return np.ascontiguousarray(ids.reshape(n_chunks, P).T.astype(np.int32))


def _quant_cols(w):
    """Per-column symmetric int8: returns (q [r, c] int8, s [c] f32)."""
    s = np.abs(w).max(axis=0) / 127.0
    s = np.maximum(s, 1e-30).astype(np.float32)
    q = np.clip(np.rint(w / s[None, :]), -127, 127).astype(np.int8)
    return q, s


def _prepare(inputs):
    """Host-side sharding. Returns (in_maps, ctx)."""
    ids = np.asarray(inputs["input_ids"])
    table = np.asarray(inputs["emb_table"], dtype=np.float32)
    w1 = np.asarray(inputs["w1"], dtype=np.float32)
    b1 = np.asarray(inputs["b1"], dtype=np.float32)
    w2 = np.asarray(inputs["w2"], dtype=np.float32)
    b2 = np.asarray(inputs["b2"], dtype=np.float32)

    B, S_in = ids.shape
    assert B == N_CORES and S_in == S, (ids.shape,)
    assert table.shape == (VOCAB, DIM)

    # new-token slice for the MLP, pre-transposed: [p, k*P + t]
    n_new = VOCAB - NEW_START
    mlp_rows = np.zeros((MLP_ROWS, DIM), dtype=np.float32)
    mlp_rows[:n_new] = table[NEW_START:]
    # When b1 == 0, relu commutes with per-row positive scales, so the MLP
    # input can ship int8 with its scales folded into the host's final
    # per-row multiply.  Otherwise fall back to bf16.
    emb_i8 = bool(np.all(b1 == 0.0))
    if emb_i8:
        se = np.maximum(np.abs(mlp_rows).max(axis=1) / 127.0, 1e-30).astype(
            np.float32
        )
        mlp_src = np.clip(np.rint(mlp_rows / se[:, None]), -127, 127).astype(np.int8)
    else:
        se = np.ones(MLP_ROWS, dtype=np.float32)
        mlp_src = mlp_rows.astype(BF16)
    mlp_rowsT = np.ascontiguousarray(
        mlp_src.reshape(MLP_ROWS, N_K_TILES, P)
        .transpose(2, 1, 0)
        .reshape(P, N_K_TILES * MLP_ROWS)
    )

    in_maps = []
    scales = []
    invs = []
    s2s = []
    for c in range(N_CORES):
        uniq, inv = np.unique(ids[c].astype(np.int64), return_inverse=True)
        packed = table[uniq]                              # [U, DIM] f32
        s = np.abs(packed).max(axis=1) / 127.0            # per-row scale
        s = np.maximum(s, 1e-30)
        q = np.clip(np.rint(packed / s[:, None]), -127, 127).astype(np.int8)
        rows = np.zeros((T_CAP, DIM), dtype=np.int8)
        rows[: uniq.size] = q
        sc = np.ones(T_CAP, dtype=np.float32)
        sc[: uniq.size] = s
        scales.append(sc)
        invs.append(inv.astype(np.int64))

        w1s = w1[:, c * SHARD_HID : (c + 1) * SHARD_HID]  # [DIM, SHARD_HID]
        w1qs, s1 = _quant_cols(w1s)
        # w1q[p, k*SHARD_HID + n] = w1qs[k*P + p, n]
        w1qp = np.ascontiguousarray(
            w1qs.reshape(N_K_TILES, P, SHARD_HID)
            .transpose(1, 0, 2)
            .reshape(P, N_K_TILES * SHARD_HID)
        )
        s1pad = np.ones(N_H_TILES * P, dtype=np.float32)
        s1pad[:SHARD_HID] = s1
        b1pad = np.zeros(N_H_TILES * P, dtype=np.float32)
        b1pad[:SHARD_HID] = b1[c * SHARD_HID : (c + 1) * SHARD_HID]
        s1b = np.ascontiguousarray(s1pad.reshape(N_H_TILES, P).T)
        b1b = np.ascontiguousarray(b1pad.reshape(N_H_TILES, P).T)

        w2s = w2[c * SHARD_HID : (c + 1) * SHARD_HID, :]  # [SHARD_HID, DIM]
        w2qs, s2 = _quant_cols(w2s)
        s2s.append(s2)
        w2pad = np.zeros((N_H_TILES * P, DIM), dtype=np.int8)
        w2pad[:SHARD_HID] = w2qs
        w2qp = np.ascontiguousarray(
            w2pad.reshape(N_H_TILES, P, DIM).transpose(1, 0, 2).reshape(P, N_H_TILES * DIM)
        )
        in_maps.append(
            {
                "ids_t": _wrap(inv.astype(np.int64), N_T_CHUNKS),
                "rows": rows,
                "mlp_rowsT": mlp_rowsT,
                "w1q": w1qp,
                "s1b": s1b,
                "b1b": b1b,
                "w2q": w2qp,
            }
        )
    ctx = dict(ids=ids, b2=b2, scales=scales, invs=invs, s2s=s2s, se=se, emb_i8=emb_i8)
    return in_maps, ctx


def _finish(results, ctx):
    ids = ctx["ids"]
    out = np.empty((N_CORES * S, DIM), dtype=np.float32)
    for c in range(N_CORES):
        # dequantize: token t's row was quantized with scale[inv[t]]
        tok_scale = ctx["scales"][c][ctx["invs"][c]]      # [S]
        out[c * S : (c + 1) * S] = (
            results[c]["out_main"].astype(np.float32) * tok_scale[:, None]
        )
    ids_flat = ids.reshape(-1).astype(np.int64)
    masked_pos = np.nonzero(ids_flat >= NEW_START)[0]
    if masked_pos.size:
        mlp = results[0]["mlp_part"].astype(np.float32) * ctx["s2s"][0][None, :]
        for c in range(1, N_CORES):
            mlp += results[c]["mlp_part"] * ctx["s2s"][c][None, :]
        n_new = mlp.shape[0]
        mlp *= ctx["se"][:n_new, None]
        mlp += ctx["b2"][None, :]
        out[masked_pos] = mlp[ids_flat[masked_pos] - NEW_START]
    return out.reshape(N_CORES, S, DIM)


def kernel(**inputs) -> np.ndarray:
    in_maps, ctx = _prepare(inputs)
    nc = build_program(emb_i8=ctx["emb_i8"])
    res = run_bass_kernel_spmd(nc, in_maps, list(range(N_CORES))).results
    return _finish(res, ctx)
